# revision 2
# baseline (speedup 1.0000x reference)
"""MoE FFN (E=8 experts, top-2 routing, SwiGLU) on 8 TRN2 NeuronCores.

Strategy (expert-parallel, per sharding hint):
  - Host computes the tiny gate (x @ Wg, 0.07% of total FLOPs), top-2 routing
    and softmax combine weights. This IS the sharding step: tokens are
    dispatched (gathered) per expert, one expert per core.
  - Each core runs the SwiGLU FFN for its expert over its routed tokens in
    bf16 (fp32 accumulation in PSUM), scales rows by the combine weight.
  - Host scatter-adds the 8 per-expert outputs back into the full [T, D]
    output (the unshard step).

Device kernel layout (per core, capacity C tokens, padded with zeros; all
matmuls bf16 with fp32 PSUM accumulation, tokens always the moving dim so
cost scales with the real token count C_comp):
  phase 1:  hT[hid, tok] = silu(W1.T @ xT) * (W3.T @ xT)
            lhsT = W1/W3 tile [128d, 128h] (stationary), rhs = xT [128d, ntok]
  phase 2:  yT[d, tok] = W2.T @ hT, scaled elementwise by the combine weight
            (host-broadcast [128, C] tile) during PSUM eviction on DVE.

Ramp design (v2): the exec-critical inputs are x chunk-0 and the first
512 w1 columns of every k-slab (2 MB).  These go out as 16 small DMAs
interleaved across BOTH HWDGE issuers (Sync and Scalar) in exact
consumption order, so a (x[k], w1[k]) pair lands every ~0.75 us and the
cold PE (one k-step of the chunk-0 k-outer sweep = ~0.85 us) never
starves.  Everything else is batched into 7 large multi-slab DMAs (3D
access patterns) issued behind the ladder in consumption order.  15
warmup matmuls keep the PE HAM-busy from ~1 us until the first pair
lands, so the clock hits 2.4 GHz during the first real group sweep and
the stream runs gap-free to the end.
"""

import os
import sys

import numpy as np

for _p in ("/opt/trn_rl_repo",):
    if os.path.isdir(_p) and _p not in sys.path:
        sys.path.insert(0, _p)

import ml_dtypes

DIM = 1024
HID = 2048
E = 8
TOPK = 2
P = 128
NCORES = 8
TBS = 512  # moving-dim token chunk for phase 1

KD = DIM // P   # 8  k-chunks over DIM
KH = HID // P   # 16 k-chunks over HID
GW = 2          # hb-group width for chunk-0 k-outer sweeps (2 => groups are
                # double-buffered in the 4-deep ph1 PSUM rotation, so a
                # group's eviction chain never stalls the next group)
NWARM = 15      # dummy 256-col matmuls bridging exec start -> first operands

BF16 = ml_dtypes.bfloat16

_KERNEL_CACHE = {}
LAST_RESULT = None  # BassKernelResults of the most recent run (for test.py)


def _chunks_for(C_comp):
    """Moving-dim chunks covering the C_comp real tokens. Chunk 0 is a full
    512 so the W1/W3 k-outer passes stretch past the weight arrivals; the
    remainder splits evenly into chunks >~250 so LDWEIGHTS (97ns) stays
    hidden behind each matmul."""
    chunks = []
    remaining = C_comp
    while remaining > 768:
        chunks.append(TBS)
        remaining -= TBS
    if remaining > 512:
        h = remaining // 2
        chunks += [remaining - h, h]
    elif remaining:
        chunks.append(remaining)
    return chunks


def _build(C, C_comp):
    import concourse.mybir as mybir
    import concourse.tile as tile
    from concourse import bacc

    f32 = mybir.dt.float32
    bf16 = mybir.dt.bfloat16
    AF = mybir.ActivationFunctionType

    chunks = _chunks_for(C_comp)

    nc = bacc.Bacc(None, target_bir_lowering=False, debug=False)

    xT = nc.declare_dram_parameter("xT", [DIM, C], bf16, isOutput=False)
    w1 = nc.declare_dram_parameter("w1", [DIM, HID], bf16, isOutput=False)
    w3 = nc.declare_dram_parameter("w3", [DIM, HID], bf16, isOutput=False)
    # db-slab-major W2: row block db holds lhsT[k, db-block] for all 16
    # k-chunks contiguously.
    w2s = nc.declare_dram_parameter("w2s", [KD * P, KH * P], bf16, isOutput=False)
    wb = nc.declare_dram_parameter("wb", [P, C], f32, isOutput=False)
    out = nc.declare_dram_parameter("out", [DIM, C], bf16, isOutput=True)

    with tile.TileContext(nc) as tc:
        with (
            tc.tile_pool(name="persist", bufs=1) as const,
            tc.tile_pool(name="psA", bufs=3, space="PSUM") as psA,
            tc.tile_pool(name="psY", bufs=1, space="PSUM") as psY,
            tc.tile_pool(name="sil", bufs=3) as sil_pool,
            tc.tile_pool(name="ysb", bufs=2) as y_pool,
        ):
            # fused slab tiles: one SBUF object per logical tensor, sliced
            # per k-slab; Tile dep-tracking is byte-range based so sliced
            # DMAs and sliced readers pair up exactly.
            xT_sb = const.tile([P, KD * C], bf16, tag="xT")
            w1_sb = const.tile([P, KD * HID], bf16, tag="w1")
            w3_sb = const.tile([P, KD * HID], bf16, tag="w3")
            w2_sb = const.tile([P, KD * KH * P], bf16, tag="w2")
            hT_sb = const.tile([P, KH * C], bf16, tag="hT")
            wb_sb = const.tile([P, C], f32, tag="wb")
            warm = const.tile([P, 256], bf16, tag="warm")
            nc.vector.memset(warm[:], 0.0)
            zb = const.tile([P, 1], f32, tag="zb")
            nc.vector.memset(zb[:], 0.0)

            def xs(k, a, b):
                return xT_sb[:, k * C + a : k * C + b]

            def w1s(k, a, b):
                return w1_sb[:, k * HID + a : k * HID + b]

            def w3s(k, a, b):
                return w3_sb[:, k * HID + a : k * HID + b]

            def w2v(db, a, b):
                return w2_sb[:, db * KH * P + a : db * KH * P + b]

            def hs(hb, a, b):
                return hT_sb[:, hb * C + a : hb * C + b]

            # PE warmup: dummy matmuls until the first real operands land.
            # Keeping the PE continuously busy from ~1us also satisfies the
            # HAM clock-ramp (full speed needs ~3.4us of uninterrupted
            # execution), so real matmuls run at 2.4GHz almost immediately.
            for _ in range(NWARM):
                wp = psA.tile([P, TBS], f32, tag="ph1", bufs=4, name="warmp")
                nc.tensor.matmul(wp[:, :256], lhsT=warm[:, :P], rhs=warm[:])

            # ── input DMA schedule ────────────────────────────────────────
            # Critical ladder: (x[k] chunk-0, w1[k] cols 0:512) pairs in
            # consumption order, alternating issuers so a pair completes
            # every ~0.75us on the two HWDGE rings.
            c0 = min(chunks[0], C)
            QW = 512
            for k in range(KD):
                e1, e2 = (nc.scalar, nc.sync) if k % 2 == 0 else (nc.sync, nc.scalar)
                e1.dma_start(out=xs(k, 0, c0), in_=xT[k * P : (k + 1) * P, :c0])
                e2.dma_start(out=w1s(k, 0, QW), in_=w1[k * P : (k + 1) * P, :QW])

            # Batched remainder (3D APs over all k-slabs), consumption order.
            def batch(eng, dst_tile, dst_block, src, a, b):
                dst = dst_tile[:, :].rearrange("p (k c) -> p k c", k=KD)[:, :, a:b]
                eng.dma_start(out=dst, in_=src[:, a:b].rearrange("(k p) c -> p k c", p=P))

            batch(nc.sync, w1_sb, HID, w1, QW, 2 * QW)          # w1 cols 512:1024
            batch(nc.scalar, w3_sb, HID, w3, 0, HID // 2)       # w3 first half
            batch(nc.sync, w1_sb, HID, w1, 2 * QW, HID)         # w1 cols 1024:2048
            batch(nc.sync, w3_sb, HID, w3, HID // 2, HID)       # w3 second half
            if c0 < C:
                batch(nc.sync, xT_sb, C, xT, c0, C)             # x remaining cols
            nc.sync.dma_start(out=wb_sb[:], in_=wb[:, :])
            batch(nc.sync, w2_sb, KH * P, w2s, 0, KH * P)       # all of w2

            # ── phase 1, chunk 0: k-outer, hb groups of GW ───────────────
            # Two passes (W1 then W3) so chunk-0 compute only depends on W1
            # + x at the start; silu(h1) staged as bf16 in slu.
            n0 = chunks[0]
            slu = const.tile([P, KH * n0], bf16, tag="slu")

            def ko_pass(w_slice, evict):
                for g in range(KH // GW):
                    phs = [
                        psA.tile([P, TBS], f32, tag="ph1", bufs=4, name=f"ph1g{j}")
                        for j in range(GW)
                    ]
                    for k in range(KD):
                        for j in range(GW):
                            hb = g * GW + j
                            nc.tensor.matmul(
                                phs[j][:, :n0],
                                lhsT=w_slice(k, hb * P, (hb + 1) * P),
                                rhs=xs(k, 0, n0),
                                start=(k == 0),
                                stop=(k == KD - 1),
                            )
                    for j in range(GW):
                        evict(g * GW + j, phs[j])

            def evict_w1(hb, ph):
                sil = sil_pool.tile([P, TBS], f32, tag="sil")
                nc.scalar.activation(sil[:, :n0], ph[:, :n0], AF.Sigmoid, bias=zb[:])
                nc.vector.tensor_mul(
                    slu[:, hb * n0 : (hb + 1) * n0], sil[:, :n0], ph[:, :n0]
                )

            def evict_w3(hb, ph):
                nc.vector.tensor_mul(
                    hs(hb, 0, n0), slu[:, hb * n0 : (hb + 1) * n0], ph[:, :n0]
                )

            ko_pass(w1s, evict_w1)
            ko_pass(w3s, evict_w3)

            # ── phase 1, remaining chunks: fused per-hid-block ph1/ph3 ───
            def mm_sweep(dst_psum, w_slice, hb, t0, n):
                for k in range(KD):
                    nc.tensor.matmul(
                        dst_psum[:, :n],
                        lhsT=w_slice(k, hb * P, (hb + 1) * P),
                        rhs=xs(k, t0, t0 + n),
                        start=(k == 0),
                        stop=(k == KD - 1),
                    )

            t0 = n0
            for n in chunks[1:]:
                for hb in range(KH):
                    ph1 = psA.tile([P, TBS], f32, tag="ph1", bufs=4)
                    ph3 = psA.tile([P, TBS], f32, tag="ph3", bufs=2)
                    mm_sweep(ph1, w1s, hb, t0, n)
                    mm_sweep(ph3, w3s, hb, t0, n)
                    # silu(h1)*h3 = sigmoid(h1)*h1*h3
                    sil = sil_pool.tile([P, TBS], f32, tag="sil")
                    sg2 = sil_pool.tile([P, TBS], f32, tag="sg2")
                    nc.scalar.activation(sil[:, :n], ph1[:, :n], AF.Sigmoid, bias=zb[:])
                    nc.vector.tensor_mul(sg2[:, :n], sil[:, :n], ph1[:, :n])
                    nc.vector.tensor_mul(hs(hb, t0, t0 + n), sg2[:, :n], ph3[:, :n])
                t0 += n

            # ── phase 2: yT[d, tok] = W2.T @ h ───────────────────────────
            # tokens as the moving dim; combine weight applied elementwise
            # against a host-broadcast [P, C] tile during PSUM eviction.
            t0 = 0
            for ci, n in enumerate(chunks):
                last_chunk = ci == len(chunks) - 1
                for db in range(KD):
                    py = psY.tile([P, TBS], f32, tag="py", bufs=2)
                    for k in range(KH):
                        nc.tensor.matmul(
                            py[:, :n],
                            lhsT=w2v(db, k * P, (k + 1) * P),
                            rhs=hs(k, t0, t0 + n),
                            start=(k == 0),
                            stop=(k == KH - 1),
                        )
                    if last_chunk and db == KD - 1:
                        # tail: split the final eviction (small second piece)
                        # so the two DMAs overlap on the two HWDGE issuers.
                        ha = (3 * n) // 4
                        ysa = y_pool.tile([P, TBS], bf16, tag="y")
                        nc.vector.tensor_mul(
                            ysa[:, :ha], py[:, :ha], wb_sb[:, t0 : t0 + ha]
                        )
                        nc.scalar.dma_start(
                            out=out[db * P : (db + 1) * P, t0 : t0 + ha],
                            in_=ysa[:, :ha],
                        )
                        ysb2 = y_pool.tile([P, TBS], bf16, tag="y")
                        nc.vector.tensor_mul(
                            ysb2[:, : n - ha], py[:, ha:n], wb_sb[:, t0 + ha : t0 + n]
                        )
                        nc.sync.dma_start(
                            out=out[db * P : (db + 1) * P, t0 + ha : t0 + n],
                            in_=ysb2[:, : n - ha],
                        )
                    else:
                        ysb = y_pool.tile([P, TBS], bf16, tag="y")
                        nc.vector.tensor_mul(ysb[:, :n], py[:, :n], wb_sb[:, t0 : t0 + n])
                        # alternate the out-DMA issuer (Scalar is idle in
                        # phase 2) so neither HWDGE stream becomes the
                        # eviction bottleneck.
                        eng = nc.sync if db % 2 == 0 else nc.scalar
                        eng.dma_start(
                            out=out[db * P : (db + 1) * P, t0 : t0 + n], in_=ysb[:, :n]
                        )
                t0 += n

    nc.compile()
    return nc


def _get_kernel(C, C_comp):
    key = (C, C_comp)
    nc = _KERNEL_CACHE.get(key)
    if nc is None:
        nc = _build(C, C_comp)
        _KERNEL_CACHE[key] = nc
    return nc


def _route(xt, Wg):
    """Host gate: returns per-expert (token_indices, combine_weights)."""
    scores = xt.astype(np.float32) @ Wg.astype(np.float32)          # [T, E]
    top2 = np.argpartition(-scores, 1, axis=1)[:, :2]               # [T, 2]
    vals = np.take_along_axis(scores, top2, axis=1)                 # [T, 2]
    vals = vals - vals.max(axis=1, keepdims=True)
    ev = np.exp(vals)
    sm = ev / ev.sum(axis=1, keepdims=True)                         # [T, 2]
    T = xt.shape[0]
    combine = np.zeros((T, E), dtype=np.float32)
    combine[np.arange(T)[:, None], top2] = sm
    idx = []
    wts = []
    for e in range(E):
        ie = np.nonzero(combine[:, e])[0]
        idx.append(ie)
        wts.append(combine[ie, e])
    return idx, wts


def _slab_w2(w):
    """[HID, DIM] -> [KD*P, KH*P] db-slab-major bf16: row block db holds, at
    [p, k*P + c], the element w[k*P + p, db*P + c]."""
    v = np.asarray(w, dtype=np.float32).reshape(KH, P, KD, P)
    return np.ascontiguousarray(v.transpose(2, 1, 0, 3).reshape(KD * P, KH * P)).astype(BF16)


def kernel(x, Wg, W1, W3, W2):
    global LAST_RESULT
    from concourse import bass_utils

    orig_shape = x.shape
    orig_dtype = x.dtype
    xt = np.ascontiguousarray(np.asarray(x, dtype=np.float32).reshape(-1, DIM))
    T = xt.shape[0]

    idx, wts = _route(xt, np.asarray(Wg, dtype=np.float32))
    max_n = max(len(i) for i in idx)
    C = max(P, -(-max_n // P) * P)
    C_comp = max(1, max_n)

    nc = _get_kernel(C, C_comp)

    W1 = np.asarray(W1)
    W3 = np.asarray(W3)
    W2 = np.asarray(W2)
    in_maps = []
    for e in range(E):
        n_e = len(idx[e])
        xT_e = np.zeros((DIM, C), dtype=BF16)
        xT_e[:, :n_e] = np.ascontiguousarray(xt[idx[e]].T).astype(BF16)
        wv_pad = np.zeros(C, dtype=np.float32)
        wv_pad[:n_e] = wts[e]
        wb_e = np.ascontiguousarray(np.broadcast_to(wv_pad, (P, C)))
        in_maps.append(
            {
                "xT": xT_e,
                "w1": W1[e].astype(BF16),
                "w3": W3[e].astype(BF16),
                "w2s": _slab_w2(W2[e]),
                "wb": wb_e,
            }
        )

    res = bass_utils.run_bass_kernel_spmd(nc, in_maps, core_ids=list(range(NCORES)))
    LAST_RESULT = res

    out = np.zeros((T, DIM), dtype=np.float32)
    for e in range(E):
        n_e = len(idx[e])
        if n_e:
            out[idx[e]] += np.asarray(
                res.results[e]["out"][:, :n_e], dtype=np.float32
            ).T
    return out.reshape(orig_shape).astype(orig_dtype, copy=False)
